# revision 93
# baseline (speedup 1.0000x reference)
"""Trainium2 Bass kernel for CNN-encoder + attention-LSTM captioner + vocab FC.

Sharding: pure data-parallel over batch (16 images -> 8 cores x 2 images).
All weights replicated; no collectives. Host slices inputs / concatenates outputs.

Structure (v2):
  - conv1 runs both images together: 64 channels of img0 on psum partitions
    0-63, img1 on 64-127 (col-tiled matmuls), with the 27-deep im2col K
    replicated into 4 PE row-groups that each cover a quarter of the image
    rows (row-tiled, concurrent streams). conv1 bias rides in K (28th row=1).
  - conv2 runs both images concurrently via 2 row-groups (img0 channels on
    partitions 0-63, img1 on 64-127) writing separate psum tiles.
  - pooling: gpsimd does horizontal max, vector does vertical max (+relu for
    conv1 via scalar_tensor_tensor); scalar evicts psum.
  - LSTM: gate order [i,f,g,o] (no perm); sigmoid(i,f) issued as soon as its
    psum slices are done so ACT overlaps the remaining gate matmuls; c update
    via two dense TTs; pstage rows selected from precomp with an identity
    matmul column pair (no per-step DMA).
  - FC: tokens of steps 0-15 are emitted interleaved into the LSTM's PE idle
    gaps; steps 16-31 run after.
"""

import os
import numpy as np

os.environ.setdefault("MYCRO_LOCAL_CACHE", "1")

HID = 640
VOCAB = 10000
T = 32
BL = 2            # local batch per core
NTOK = T * BL     # 64
NCORES = 8
CH = 1000

F32 = None  # set lazily (mybir.dt.float32)


class _PhaseExit(Exception):
    def __init__(self, tc):
        self.tc = tc

_NC_CACHE = {}


def build_bass(upto=None):
    import os
    upto = upto or os.environ.get("KERNEL_UPTO", "all")
    import concourse.bass as bass
    from concourse import bacc
    import concourse.tile_sem_assignment as tsa
    # Cap HWDGE sem lanes so pool-transition fan-ins stay under the
    # per-instruction sync-wait slot limits in walrus codegen.
    tsa.NUM_HWDGE_SEMS = 4
    import concourse.mybir as mybir
    import concourse.tile as tile
    from concourse.masks import make_identity

    f32 = mybir.dt.float32
    i32 = mybir.dt.int32
    AF = mybir.ActivationFunctionType
    ALU = mybir.AluOpType
    AX = mybir.AxisListType

    nc = bacc.Bacc(None)
    bf16 = mybir.dt.bfloat16

    def mm(out, lhsT, rhs, **kw):
        nc.tensor.matmul(out=out, lhsT=lhsT, rhs=rhs, **kw)

    # ---------------- DRAM parameters ----------------
    # img: [im, group, 28 rows (27 im2col K + ones), 56*224]
    img_d = nc.declare_dram_parameter("img", [BL, 4, 28, 56 * 224], bf16, isOutput=False)
    caps_d = nc.declare_dram_parameter("caps", [NTOK, 1], i32, isOutput=False)
    w1q_d = nc.declare_dram_parameter("w1q", [128, 64], bf16, isOutput=False)
    cb2_d = nc.declare_dram_parameter("cb2t", [128, 1], f32, isOutput=False)
    w2p9_d = nc.declare_dram_parameter("w2p9", [9, 128, 128], bf16, isOutput=False)
    w3t9_d = nc.declare_dram_parameter("w3t9", [9, 128, 256], bf16, isOutput=False)
    w4t9_d = nc.declare_dram_parameter("w4t9", [9, 2, 128, 512], bf16, isOutput=False)
    cb3_d = nc.declare_dram_parameter("cb3t", [128, 2], f32, isOutput=False)
    cb4_d = nc.declare_dram_parameter("cb4t", [128, 4], f32, isOutput=False)
    encw_d = nc.declare_dram_parameter("encwt", [4, 128, HID], f32, isOutput=False)
    encb_d = nc.declare_dram_parameter("encbt", [128, 5], f32, isOutput=False)
    emb_d = nc.declare_dram_parameter("emb", [VOCAB, HID], bf16, isOutput=False)
    attnw_d = nc.declare_dram_parameter("attnwt", [10, 128, HID], bf16, isOutput=False)
    attnb_d = nc.declare_dram_parameter("attnb", [1, HID], bf16, isOutput=False)
    wih_d = nc.declare_dram_parameter("wiht", [10, 128, 4 * HID], bf16, isOutput=False)
    whh_d = nc.declare_dram_parameter("whht", [5, 128, 4 * HID], bf16, isOutput=False)
    bgate_d = nc.declare_dram_parameter("bgate", [1, 4 * HID], bf16, isOutput=False)
    fcw_d = nc.declare_dram_parameter("fcwt", [5, 128, VOCAB], bf16, isOutput=False)
    fcb_d = nc.declare_dram_parameter("fcb", [1, VOCAB], bf16, isOutput=False)
    bsel_d = nc.declare_dram_parameter("bsel", [BL, NTOK], f32, isOutput=False)
    logits_d = nc.declare_dram_parameter("logits", [BL, T, VOCAB], f32, isOutput=True)

    try:
      with tile.TileContext(nc) as tc:
        # ---------------- persistent constants ----------------
        cpool = tc.alloc_tile_pool(name="const", bufs=1)
        # pool for all DMA-written tiles: never released mid-kernel so that
        # SBUF zone reuse never makes compute ops wait on DMA queue sems
        dmapool = tc.alloc_tile_pool(name="dmat", bufs=1)
        spool = tc.alloc_tile_pool(name="seq", bufs=1)
        prepool = tc.alloc_tile_pool(name="predma", bufs=1)
        ident = cpool.tile([128, 128], f32)
        make_identity(nc, ident[:, :])
        identb = cpool.tile([128, 128], bf16)
        make_identity(nc, identb[:, :])
        identb64 = cpool.tile([64, 64], bf16)
        make_identity(nc, identb64[:, :])
        ones64 = cpool.tile([1, 64], bf16)
        nc.gpsimd.memset(ones64[:, :], 1.0)
        bsel_sb = dmapool.tile([BL, NTOK], f32)
        nc.sync.dma_start(out=bsel_sb[:, :], in_=bsel_d[:, :])
        feat_sb = cpool.tile([128, 4, BL], f32)   # feat.T, K-chunked [128,4] per img

        w1q_sb = dmapool.tile([128, 64], bf16)
        nc.sync.dma_start(out=w1q_sb[:, :], in_=w1q_d[:, :])
        cb2_sb = dmapool.tile([128, 1], f32)
        nc.sync.dma_start(out=cb2_sb[:, :], in_=cb2_d[:, :])
        w2p_sb = dmapool.tile([128, 9, 128], bf16)
        nc.sync.dma_start(out=w2p_sb[:, :, :], in_=w2p9_d[:, :, :].rearrange("t p o -> p t o"))
        w3_sb = dmapool.tile([128, 9, 256], bf16)
        nc.sync.dma_start(out=w3_sb[:, :, :], in_=w3t9_d[:, :, :].rearrange("t p o -> p t o"))
        cb3_sb = dmapool.tile([128, 2], f32)
        nc.sync.dma_start(out=cb3_sb[:, :], in_=cb3_d[:, :])
        cb4_sb = dmapool.tile([128, 4], f32)
        nc.sync.dma_start(out=cb4_sb[:, :], in_=cb4_d[:, :])

        # ---------------- conv1 + pool1: both images together ----------------
        ppool = tc.alloc_tile_pool(name="pair", bufs=1)
        x2pair = ppool.tile([128, 114, 114], bf16)   # p0-63 img0, p64-127 img1
        nc.gpsimd.memset(x2pair[:, :, :], 0.0)

        c1psum = tc.alloc_tile_pool(name="c1p", bufs=2, space="PSUM")
        c1pool = tc.alloc_tile_pool(name="c1", bufs=4)
        convdma = tc.alloc_tile_pool(name="convdma", bufs=1)
        post_ct = 0
        for ch in range(2):
            rh = []
            for im in range(BL):
                t_ = convdma.tile([128, 28 * 224], bf16, tag=f"rh{im}", bufs=1)
                for g in range(4):
                    nc.sync.dma_start(
                        out=t_[32 * g:32 * g + 28, :],
                        in_=img_d[im, g, :, ch * 28 * 224:(ch + 1) * 28 * 224])
                rh.append(t_.rearrange("p (r x) -> p r x", x=224))
            for (r0, nr) in ((0, 8), (8, 8), (16, 8), (24, 4)):
                for g in range(4):
                    ns = nr // 2
                    ps = c1psum.tile([128, ns, 448], f32,
                                     padded_shape=[128, ns, 512], tag="ps")
                    for s in range(ns):
                        rr = r0 + 2 * s
                        for im in range(BL):
                            mm(out=ps[64 * im:64 * im + 64, s, :],
                               lhsT=w1q_sb[32 * g:32 * g + 28, :],
                               rhs=rh[im][32 * g:32 * g + 28, rr:rr + 2, :],
                               start=True, stop=True,
                               tile_position=(32 * g, 64 * im))
                    a1 = c1pool.tile([128, nr, 224], bf16, tag="a1")
                    a1v = a1.rearrange("p (s r) x -> p s r x", r=2)
                    psv = ps.rearrange("p s (r x) -> p s r x", x=224)
                    if post_ct % 10 == 9:
                        nc.vector.tensor_copy(out=a1v, in_=psv)
                    else:
                        nc.scalar.copy(a1v, psv)
                    post_ct += 1
                    t1 = c1pool.tile([128, nr, 112], bf16, tag="t1")
                    nc.vector.tensor_tensor(
                        out=t1[:, :, :],
                        in0=a1[:, :, 0:224:2], in1=a1[:, :, 1:224:2],
                        op=ALU.max)
                    oy = 28 * g + 14 * ch + r0 // 2
                    # vertical pool + relu (bias already added via K row)
                    nc.vector.scalar_tensor_tensor(
                        out=x2pair[:, oy + 1:oy + 1 + ns, 1:113],
                        in0=t1[:, 0:nr:2, :], scalar=0.0, in1=t1[:, 1:nr:2, :],
                        op0=ALU.max, op1=ALU.max)
        convdma.release()
        c1pool.release()
        c1psum.release()

        # ---- prefetch the remaining weights while conv2-conv4 compute ----
        # (emitted after the image DMAs so they don't delay conv1's inputs)
        w4ms0 = []
        for m in range(4):
            w4m_ = dmapool.tile([128, 2, 9, 128], bf16, tag="w4m", bufs=4)
            for k2 in range(2):
                nc.sync.dma_start(
                    out=w4m_[:, k2, :, :],
                    in_=w4t9_d[:, k2, :, 128 * m:128 * (m + 1)].rearrange(
                        "t p o -> p t o"))
            w4ms0.append(w4m_)
        encw_sb = prepool.tile([128, 4, HID], f32)
        nc.sync.dma_start(out=encw_sb[:, :, :],
                          in_=encw_d[:, :, :].rearrange("k p o -> p k o"))
        encb_sb = prepool.tile([128, 5], f32)
        nc.sync.dma_start(out=encb_sb[:, :], in_=encb_d[:, :])
        idx_sb = prepool.tile([NTOK, 1], i32)
        nc.sync.dma_start(out=idx_sb[:, :], in_=caps_d[:, :])
        e_sb = prepool.tile([NTOK, HID], bf16)
        nc.gpsimd.indirect_dma_start(
            out=e_sb[:, :], out_offset=None,
            in_=emb_d[:, :],
            in_offset=bass.IndirectOffsetOnAxis(ap=idx_sb[:, :1], axis=0),
        )
        attnw_sb = prepool.tile([128, 10, HID], bf16)
        nc.sync.dma_start(out=attnw_sb[:, :, :],
                          in_=attnw_d[:, :, :].rearrange("k p o -> p k o"))
        attnb_sb = prepool.tile([1, HID], bf16)
        nc.sync.dma_start(out=attnb_sb[:, :], in_=attnb_d[:, :])
        bgate_sb = prepool.tile([1, 4 * HID], bf16, tag="bgate", bufs=1)
        nc.sync.dma_start(out=bgate_sb[:, :], in_=bgate_d[:, :])
        wihks = []
        for k in range(10):
            wih_k = prepool.tile([128, 4 * HID], bf16, tag="wihk", bufs=4)
            nc.sync.dma_start(out=wih_k[:, :], in_=wih_d[k, :, :])
            wihks.append(wih_k)
        whh_sb = dmapool.tile([128, 5, 4 * HID], bf16)
        nc.sync.dma_start(out=whh_sb[:, :, :],
                          in_=whh_d[:, :, :].rearrange("k p o -> p k o"))

        # ---------------- conv2 + pool2: both images via 2 row-groups ----------------
        x3 = []
        for im in range(BL):
            x3_ = ppool.tile([128, 58, 58], bf16, tag=f"x3_{im}")
            nc.gpsimd.memset(x3_[:, :, :], 0.0)
            x3.append(x3_)
        c2psum = tc.alloc_tile_pool(name="c2p", bufs=2, space="PSUM")
        c2pool = tc.alloc_tile_pool(name="c2", bufs=3)
        for tl in range(14):  # 8 output rows per tile
            ps2_0 = c2psum.tile([128, 2, 448], f32, padded_shape=[128, 2, 512],
                                tag="ps0")
            ps2_1 = c2psum.tile([128, 2, 448], f32, padded_shape=[128, 2, 512],
                                tag="ps1")
            pss = [ps2_0, ps2_1]
            for s in range(2):
                y0 = tl * 8 + s * 4
                for ky in range(3):
                    for kx in range(3):
                        tap = ky * 3 + kx
                        for im in range(BL):
                            mm(out=pss[im][:, s, :],
                               lhsT=w2p_sb[64 * im:64 * im + 64, tap, :],
                               rhs=x2pair[64 * im:64 * im + 64,
                                          y0 + ky:y0 + ky + 4, kx:kx + 112],
                               start=(tap == 0), stop=(tap == 8),
                               tile_position=(64 * im, 0))
            for im in range(BL):
                a2 = c2pool.tile([128, 8, 112], bf16, tag="a2")
                nc.scalar.activation(
                    a2.rearrange("p (a y) x -> p a y x", a=2),
                    pss[im].rearrange("p a (y x) -> p a y x", x=112),
                    AF.Relu, bias=cb2_sb[:, 0:1])
                t2 = c2pool.tile([128, 8, 56], bf16, tag="t2")
                nc.vector.tensor_tensor(
                    out=t2[:, :, :], in0=a2[:, :, 0:112:2], in1=a2[:, :, 1:112:2],
                    op=ALU.max)
                nc.vector.tensor_tensor(
                    out=x3[im][:, tl * 4 + 1:tl * 4 + 5, 1:57],
                    in0=t2[:, 0:8:2, :], in1=t2[:, 1:8:2, :],
                    op=ALU.max)
        c2psum.release()
        c2pool.release()

        # ---------------- conv3 + conv4, per image ----------------
        w4ms = [w4ms0, None]
        for im in range(BL):
            if im == 1:
                w4ms1 = []
                for m in range(4):
                    w4m_ = dmapool.tile([128, 2, 9, 128], bf16, tag="w4m", bufs=4)
                    for k2 in range(2):
                        nc.sync.dma_start(
                            out=w4m_[:, k2, :, :],
                            in_=w4t9_d[:, k2, :, 128 * m:128 * (m + 1)].rearrange(
                                "t p o -> p t o"))
                    w4ms1.append(w4m_)
                w4ms[1] = w4ms1
            ipool = tc.alloc_tile_pool(name=f"img{im}", bufs=1)
            # ---- conv3 (128->256) K=128, bias via ACT evict, pool -> x4_pad ----
            x4_pad = ipool.tile([128, 2, 30, 30], bf16)
            nc.gpsimd.memset(x4_pad[:, :, :, :], 0.0)
            c3psum = tc.alloc_tile_pool(name=f"c3p_{im}", bufs=3, space="PSUM")
            c3pool = tc.alloc_tile_pool(name=f"c3_{im}", bufs=2)
            for m in range(2):
                for tl in range(7):  # 8 output rows per tile
                    ps = c3psum.tile([128, 448], f32, padded_shape=[128, 512], tag="ps")
                    y0 = tl * 8
                    for ky in range(3):
                        for kx in range(3):
                            tap = ky * 3 + kx
                            rhs = x3[im][:, y0 + ky:y0 + ky + 8, kx:kx + 56]
                            mm(
                                out=ps[:, :],
                                lhsT=w3_sb[:, tap, 128 * m:128 * (m + 1)],
                                rhs=rhs,
                                start=(tap == 0), stop=(tap == 8),
                            )
                    a3 = c3pool.tile([128, 8, 56], bf16, tag="a3")
                    nc.scalar.activation(
                        a3[:, :, :],
                        ps.rearrange("p (y x) -> p y x", x=56),
                        AF.Relu, bias=cb3_sb[:, m:m + 1])
                    t3 = c3pool.tile([128, 8, 28], bf16, tag="t3")
                    nc.vector.tensor_tensor(
                        out=t3[:, :, :], in0=a3[:, :, 0:56:2], in1=a3[:, :, 1:56:2],
                        op=ALU.max)
                    nc.vector.tensor_tensor(
                        out=x4_pad[:, m, tl * 4 + 1:tl * 4 + 5, 1:29],
                        in0=t3[:, 0:8:2, :], in1=t3[:, 1:8:2, :],
                        op=ALU.max)
            c3psum.release()
            c3pool.release()

            # ---- conv4 (256->512) K=256 (2 chunks), no pool; mean via accum_out ----
            c4psum = tc.alloc_tile_pool(name=f"c4p_{im}", bufs=3, space="PSUM")
            c4pool = tc.alloc_tile_pool(name=f"c4_{im}", bufs=2)
            msum = ipool.tile([128, 4, 2], f32)
            for m in range(4):
                w4m = w4ms[im][m]
                ps = c4psum.tile([128, 2, 392], f32, padded_shape=[128, 2, 512], tag="ps")
                for s in range(2):
                    y0 = s * 14
                    first = True
                    for ky in range(3):
                        for kx in range(3):
                            tap = ky * 3 + kx
                            for k2 in range(2):
                                rhs = x4_pad[:, k2, y0 + ky:y0 + ky + 14, kx:kx + 28]
                                mm(
                                    out=ps[:, s, :],
                                    lhsT=w4m[:, k2, tap, :],
                                    rhs=rhs,
                                    start=first, stop=(tap == 8 and k2 == 1),
                                )
                                first = False
                a4 = c4pool.tile([128, 2, 392], bf16, tag="a4")
                for s in range(2):
                    nc.scalar.activation(a4[:, s, :], ps[:, s, :], AF.Relu,
                                         bias=cb4_sb[:, m:m + 1],
                                         accum_out=msum[:, m, s:s + 1])
            c4psum.release()
            c4pool.release()
            # feat.T[:, m] = (msum[:,m,0] + msum[:,m,1]) / 784
            tmpf = ipool.tile([128, 4], f32)
            nc.vector.tensor_tensor(out=tmpf[:, :], in0=msum[:, :, 0], in1=msum[:, :, 1],
                                    op=ALU.add)
            nc.vector.tensor_scalar_mul(feat_sb[:, :, im], tmpf[:, :], 1.0 / 784.0)
            ipool.release()
        ppool.release()

        if upto == "conv":
            raise _PhaseExit(tc)

        # ---------------- encoder linear: memory.T = enc_w @ feat.T + enc_b ----------------
        scpool = tc.alloc_tile_pool(name="scratch", bufs=1)

        p1psum = tc.alloc_tile_pool(name="p1ps", bufs=1, space="PSUM")

        def emit_warm(pool, n=2):
            # dummy matmuls that keep the PE's HAM activity window busy
            # through serial/sparse stretches so real matmuls stay at 2.4 GHz
            wt = pool.tile([BL, 512], f32, tag="warm")
            for w in range(n):
                mm(out=wt[:, :], lhsT=identb64[:, 2 * w:2 * w + 2],
                   rhs=e_sb[:, 0:512], start=True, stop=True)

        memT_ps = p1psum.tile([128, 5, BL], f32)
        for m in range(5):
            for k in range(4):
                nc.tensor.matmul(
                    out=memT_ps[:, m, :],
                    lhsT=encw_sb[:, k, 128 * m:128 * (m + 1)],
                    rhs=feat_sb[:, k, :],
                    start=(k == 0), stop=(k == 3),
                )
        memT_sb = spool.tile([128, 5, BL], f32)
        for m in range(5):
            nc.vector.tensor_scalar_add(memT_sb[:, m, :], memT_ps[:, m, :],
                                        encb_sb[:, m:m + 1])
        emit_warm(p1psum)
        # memory non-transposed [2, 640]
        mem_ps = p1psum.tile([BL, HID], f32)
        for m in range(5):
            nc.tensor.transpose(out=mem_ps[:, 128 * m:128 * (m + 1)],
                                in_=memT_sb[:, m, :], identity=ident[:, :])
        mem_sb = scpool.tile([BL, HID], f32)
        nc.scalar.copy(mem_sb[:, :], mem_ps[:, :])
        emit_warm(p1psum)

        # memory broadcast to all tokens [64, 640] via bsel matmul
        mexp_ps = p1psum.tile([NTOK, HID], f32)
        for n in range(2):
            sl = slice(512 * n, min(HID, 512 * (n + 1)))
            nc.tensor.matmul(out=mexp_ps[:, sl], lhsT=bsel_sb[:, :], rhs=mem_sb[:, sl],
                             start=True, stop=True)
        mexp_sb = scpool.tile([NTOK, HID], f32)
        nc.scalar.copy(mexp_sb[:, :], mexp_ps[:, :])
        emit_warm(p1psum)
        p1psum.release()
        p1bpsum = tc.alloc_tile_pool(name="p1bps", bufs=1, space="PSUM")

        # ---------------- embeddings gather + fusedT ----------------
        # fusedT [128, 10, 64]: chunks 0-4 = e.T ; 5-9 = memory.T broadcast
        fusedT_pse = p1bpsum.tile([128, 5, NTOK], bf16)
        for k in range(5):
            nc.tensor.transpose(out=fusedT_pse[:, k, :],
                                in_=e_sb[:, 128 * k:128 * (k + 1)],
                                identity=identb[0:64, 0:64])
        fusedT_psm = p1bpsum.tile([128, 5, NTOK], f32)
        for m in range(5):
            nc.tensor.matmul(out=fusedT_psm[:, m, :],
                             lhsT=mem_sb[:, 128 * m:128 * (m + 1)],
                             rhs=bsel_sb[:, :], start=True, stop=True)
        fusedT_sb = spool.tile([128, 10, NTOK], bf16)
        nc.scalar.copy(fusedT_sb[:, 0:5, :], fusedT_pse[:, :, :])
        nc.scalar.copy(fusedT_sb[:, 5:10, :], fusedT_psm[:, :, :])

        # ---------------- attention (batched over all tokens) ----------------
        attn_ps = p1bpsum.tile([NTOK, HID], f32)
        for n in range(2):
            sl = slice(512 * n, min(HID, 512 * (n + 1)))
            for k in range(10):
                mm(out=attn_ps[:, sl], lhsT=fusedT_sb[:, k, :],
                   rhs=attnw_sb[:, k, sl], start=(k == 0), stop=False)
            mm(out=attn_ps[:, sl], lhsT=ones64[:, :],
               rhs=attnb_sb[:, sl], start=False, stop=True)
        # softmax over free dim, then context = softmax * memory
        emit_warm(p1bpsum, 3)
        nmx_sb = scpool.tile([NTOK, 1], f32)
        nc.vector.reduce_max(out=nmx_sb[:, :], in_=attn_ps[:, :], axis=AX.X,
                             negate=True)
        ex_sb = scpool.tile([NTOK, HID], f32)
        ssum_sb = scpool.tile([NTOK, 1], f32)
        nc.scalar.activation(ex_sb[:, :], attn_ps[:, :], AF.Exp,
                             bias=nmx_sb[:, 0:1], accum_out=ssum_sb[:, 0:1])
        rcp_sb = scpool.tile([NTOK, 1], f32)
        nc.vector.reciprocal(rcp_sb[:, :], ssum_sb[:, :])
        ctx_sb = scpool.tile([NTOK, HID], bf16)
        emit_warm(p1bpsum, 3)
        nc.vector.tensor_scalar_mul(ctx_sb[:, :], ex_sb[:, :], rcp_sb[:, 0:1])
        nc.vector.tensor_tensor(out=ctx_sb[:, :], in0=ctx_sb[:, :], in1=mexp_sb[:, :],
                                op=ALU.mult)
        ctxT_ps = p1bpsum.tile([128, 5, NTOK], bf16)
        for k in range(5):
            nc.tensor.transpose(out=ctxT_ps[:, k, :],
                                in_=ctx_sb[:, 128 * k:128 * (k + 1)],
                                identity=identb[0:64, 0:64])
        ctxT_sb = spool.tile([128, 5, NTOK], bf16)
        nc.scalar.copy(ctxT_sb[:, :, :], ctxT_ps[:, :, :])
        p1bpsum.release()
        scpool.release()

        # ---------------- gates precompute: xin @ w_ih.T + (b_ih+b_hh) ----------------
        p2psum = tc.alloc_tile_pool(name="p2ps", bufs=1, space="PSUM")
        P_ps = p2psum.tile([NTOK, 4 * HID], f32)
        for k in range(10):
            wih_k = wihks[k]
            lhsT = fusedT_sb[:, k, :] if k < 5 else ctxT_sb[:, k - 5, :]
            for n in range(5):
                sl = slice(512 * n, 512 * (n + 1))
                mm(out=P_ps[:, sl], lhsT=lhsT, rhs=wih_k[:, sl],
                   start=(k == 0), stop=False)
        for n in range(5):
            sl = slice(512 * n, 512 * (n + 1))
            mm(out=P_ps[:, sl], lhsT=ones64[:, :], rhs=bgate_sb[:, sl],
               start=False, stop=True)
        precomp_sb = spool.tile([NTOK, 4 * HID], bf16)
        nc.scalar.copy(precomp_sb[:, :], P_ps[:, :])
        p2psum.release()
        prepool.release()

        if upto == "pre":
            raise _PhaseExit(tc)
        # ---------------- LSTM recurrence + interleaved first-half FC ----------------
        outsT_sb = spool.tile([128, 5, NTOK], bf16)   # h.T for every step
        sigf_sb = spool.tile([BL, HID], f32)
        g_sb = spool.tile([BL, HID], f32)
        sigi_sb = spool.tile([BL, HID], f32)
        sigo_sb = spool.tile([BL, HID], f32)
        # transposed-state tiles: the whole c/h chain runs as [128, 5, 2]
        cT_sb = spool.tile([128, 5, BL], f32)
        nc.vector.memset(cT_sb[:, :, :], 0.0)         # c_{-1} = 0
        tmpA_sb = spool.tile([128, 5, BL], f32)
        tmpB_sb = spool.tile([128, 5, BL], f32)
        thcT_sb = spool.tile([128, 5, BL], f32)

        # FC weight stream: allocate + DMA before the LSTM so transfers overlap it
        fcpsum = tc.alloc_tile_pool(name="fc_ps", bufs=1, space="PSUM")
        lpsum = tc.alloc_tile_pool(name="lstm_ps", bufs=1, space="PSUM")
        fcpool = tc.alloc_tile_pool(name="fcw", bufs=1)
        fcb_sb = fcpool.tile([1, VOCAB], bf16)
        nc.sync.dma_start(out=fcb_sb[:, :], in_=fcb_d[:, :])
        fws = []
        for j in range(VOCAB // CH):
            fw = fcpool.tile([128, 5, CH], bf16, tag="fw", bufs=VOCAB // CH)
            nc.sync.dma_start(out=fw[:, :, :],
                              in_=fcw_d[:, :, CH * j:CH * (j + 1)].rearrange(
                                  "k p o -> p k o"))
            fws.append(fw)

        def emit_fc_block(j, s, half, pool):
            toks = slice(32 * half, 32 * half + 32)
            fw = fws[j]
            ps = pool.tile([32, 500], f32, padded_shape=[32, 512], tag="ps")
            for k in range(5):
                mm(out=ps[:, :], lhsT=outsT_sb[:, k, toks],
                   rhs=fw[:, k, 500 * s:500 * (s + 1)],
                   start=(k == 0), stop=False)
            mm(out=ps[:, :], lhsT=ones64[:, 0:32],
               rhs=fcb_sb[:, CH * j + 500 * s:CH * j + 500 * (s + 1)],
               start=False, stop=True)
            lo = spool.tile([32, 500], f32, tag="lo", bufs=2)
            nc.scalar.copy(lo[:, :], ps[:, :])
            nc.sync.dma_start(
                out=logits_d[:, 16 * half:16 * half + 16,
                             CH * j + 500 * s:CH * j + 500 * (s + 1)]
                    .rearrange("b t v -> t b v"),
                in_=lo[:, :],
            )

        fc_blocks = [(j, s) for j in range(VOCAB // CH) for s in range(2)]
        NB = len(fc_blocks)

        # gate column layout (host-permuted): f=[0:H) g=[H:2H) i=[2H:3H) o=[3H:4H)
        # psum split: gFG = [0:2H), gIO = [2H:4H) — separate tiles keep the f/g
        # activation reads from serializing against the i/o matmuls
        # (tile-granular WAR deps).
        FG_SLICES = [(0, 512), (512, 1024), (1024, 1280)]
        IO_SLICES = [(0, 512), (512, 1024), (1024, 1280)]

        def emit_pstage_mms(t, g_ps, base, slices):
            # the precomp-select matmuls don't depend on h(t-1): emitting them
            # first lets them run during the previous step's nonlinearity tail
            for (lo, hi) in slices:
                mm(out=g_ps[:, lo:hi], lhsT=identb64[:, 2 * t:2 * t + 2],
                   rhs=precomp_sb[:, base + lo:base + hi],
                   start=True, stop=(t == 0))

        def emit_whh_mms(t, g_ps, base, slices):
            for (lo, hi) in slices:
                for k in range(5):
                    mm(
                        out=g_ps[:, lo:hi],
                        lhsT=outsT_sb[:, k, 2 * (t - 1):2 * t],
                        rhs=whh_sb[:, k, base + lo:base + hi],
                        start=False, stop=(k == 4),
                    )

        for t in range(T):
            gfg_ps = lpsum.tile([BL, 2 * HID], f32, tag="gfg")
            gio_ps = lpsum.tile([BL, 2 * HID], f32, tag="gio")
            trT_ps = lpsum.tile([128, 4, 5, BL], f32, tag="trT")
            emit_pstage_mms(t, gfg_ps, 0, FG_SLICES)
            emit_pstage_mms(t, gio_ps, 2 * HID, IO_SLICES)
            if t > 0:
                emit_whh_mms(t, gfg_ps, 0, FG_SLICES)
            nc.scalar.activation(sigf_sb[:, :], gfg_ps[:, 0:HID], AF.Sigmoid)
            nc.scalar.activation(g_sb[:, :], gfg_ps[:, HID:2 * HID], AF.Tanh)
            if t > 0:
                emit_whh_mms(t, gio_ps, 2 * HID, IO_SLICES)
            nc.scalar.activation(sigi_sb[:, :], gio_ps[:, 0:HID], AF.Sigmoid)
            nc.scalar.activation(sigo_sb[:, :], gio_ps[:, HID:2 * HID],
                                 AF.Sigmoid)
            # transpose sig_f, g, sig_i into [128, 5, 2] chunks; sig_o's
            # transposes are emitted after the c-chain so the chain's reads
            # of trT (tile-granular deps) don't wait on them
            for j, src in ((1, sigf_sb[:, :]), (2, g_sb[:, :]),
                           (0, sigi_sb[:, :])):
                for k in range(5):
                    nc.tensor.transpose(out=trT_ps[:, j, k, :],
                                        in_=src[:, 128 * k:128 * k + 128],
                                        identity=ident[0:2, 0:2])
            # c = sig_f*c_prev + sig_i*g ; h = sig_o * tanh(c)  (all [128,5,2])
            gT_sb = spool.tile([128, 5, BL], f32, tag="gT", bufs=2)
            nc.vector.tensor_tensor(out=tmpA_sb[:, :, :], in0=trT_ps[:, 1, :, :],
                                    in1=cT_sb[:, :, :], op=ALU.mult)
            nc.vector.tensor_copy(out=gT_sb[:, :, :], in_=trT_ps[:, 2, :, :])
            nc.vector.tensor_tensor(out=tmpB_sb[:, :, :], in0=trT_ps[:, 0, :, :],
                                    in1=gT_sb[:, :, :], op=ALU.mult)
            nc.vector.tensor_tensor(out=cT_sb[:, :, :], in0=tmpA_sb[:, :, :],
                                    in1=tmpB_sb[:, :, :], op=ALU.add)
            nc.scalar.activation(thcT_sb[:, :, :], cT_sb[:, :, :], AF.Tanh)
            for k in range(5):
                nc.tensor.transpose(out=trT_ps[:, 3, k, :],
                                    in_=sigo_sb[:, 128 * k:128 * k + 128],
                                    identity=ident[0:2, 0:2])
            nc.vector.tensor_tensor(out=outsT_sb[:, :, 2 * t:2 * t + 2],
                                    in0=trT_ps[:, 3, :, :],
                                    in1=thcT_sb[:, :, :], op=ALU.mult)
            if t >= 16:
                # interleave first-half FC blocks into the PE tail
                b0 = (t - 16) * NB // 16
                b1 = (t - 15) * NB // 16
                for b in range(b0, b1):
                    emit_fc_block(*fc_blocks[b], half=0, pool=fcpsum)
            elif t > 0:
                # warm-keeper matmuls: keep the PE's HAM activity window busy
                # through the nonlinearity tail so the next step's matmuls
                # don't run at the cold half-clock
                wps = fcpsum.tile([32, 500], f32, padded_shape=[32, 512],
                                  tag="ps")
                for w in range(3):
                    mm(out=wps[0:2, :], lhsT=identb64[:, 0:2],
                       rhs=precomp_sb[:, 500 * w:500 * (w + 1)],
                       start=True, stop=True)
        lpsum.release()

        if upto == "lstm":
            raise _PhaseExit(tc)
        # ---------------- second-half FC to vocab ----------------
        fcpsum2 = tc.alloc_tile_pool(name="fc_ps2", bufs=4, space="PSUM")
        for b in range(NB):
            emit_fc_block(*fc_blocks[b], half=1, pool=fcpsum2)
        fcpsum2.release()
        fcpsum.release()
        fcpool.release()
        spool.release()
        dmapool.release()
        cpool.release()
    except _PhaseExit:
        pass

    nc.finalize()
    return nc


def _prep_shared(inputs):
    """Host-side weight layout prep (shared across cores)."""
    import ml_dtypes
    bf = ml_dtypes.bfloat16
    f = np.float32
    w1 = inputs["cw1"].astype(f)
    w1b = w1.transpose(2, 3, 1, 0).reshape(27, 64)
    w1q = np.zeros((128, 64), f)
    for g in range(4):
        w1q[32 * g:32 * g + 27] = w1b
        w1q[32 * g + 27] = inputs["cb1"].astype(f)
    cb2t = inputs["cb2"].astype(f).reshape(128, 1).copy()
    w2t9 = inputs["cw2"].astype(f).transpose(2, 3, 1, 0).reshape(9, 64, 128)
    w2p9 = np.concatenate([w2t9, w2t9], axis=1)   # [9, 128, 128]
    w3t9 = inputs["cw3"].astype(f).transpose(2, 3, 1, 0).reshape(9, 128, 256)
    w4t9 = inputs["cw4"].astype(f).transpose(2, 3, 1, 0).reshape(9, 2, 128, 512)
    cb3t = inputs["cb3"].astype(f).reshape(2, 128).T.copy()
    cb4t = inputs["cb4"].astype(f).reshape(4, 128).T.copy()
    encwt = inputs["enc_w"].astype(f).T.reshape(4, 128, HID).copy()
    encbt = inputs["enc_b"].astype(f).reshape(5, 128).T.copy()
    attnwt = inputs["attn_w"].astype(f).T.reshape(10, 128, HID).copy()
    attnb = inputs["attn_b"].astype(f)[None, :]
    # gate column order [f, g, i, o]: f and g (the c-update inputs) first so
    # their activations start after only the first psum tile's matmuls
    perm = np.concatenate([np.arange(HID, 3 * HID), np.arange(0, HID),
                           np.arange(3 * HID, 4 * HID)])
    wih = inputs["w_ih"].astype(f)[perm]
    whh = inputs["w_hh"].astype(f)[perm]
    wiht = wih.T.reshape(10, 128, 4 * HID).copy()
    whht = whh.T.reshape(5, 128, 4 * HID).copy()
    bgate = (inputs["b_ih"].astype(f) + inputs["b_hh"].astype(f))[perm][None, :]
    fcwt = inputs["fc_w"].astype(f).T.reshape(5, 128, VOCAB).copy()
    fcb = inputs["fc_b"].astype(f)[None, :]
    bsel = np.zeros((BL, NTOK), f)
    for p in range(NTOK):
        bsel[p % BL, p] = 1.0
    return dict(w1q=w1q.astype(bf), cb2t=cb2t,
                w2p9=w2p9.astype(bf), w3t9=w3t9.astype(bf), w4t9=w4t9.astype(bf),
                cb3t=cb3t, cb4t=cb4t, encwt=encwt, encbt=encbt,
                attnwt=attnwt.astype(bf), attnb=attnb.astype(bf),
                wiht=wiht.astype(bf), whht=whht.astype(bf), bgate=bgate.astype(bf),
                fcwt=fcwt.astype(bf), fcb=fcb.astype(bf), bsel=bsel,
                emb=inputs["emb"].astype(f).astype(bf))


def _make_in_maps(inputs):
    shared = _prep_shared(inputs)
    images = np.asarray(inputs["images"], np.float32)
    captions = np.asarray(inputs["captions"])

    import ml_dtypes
    imgp = np.zeros((16, 3, 226, 226), np.float32)
    imgp[:, :, 1:225, 1:225] = images
    s = imgp.strides
    win = np.lib.stride_tricks.as_strided(
        imgp, shape=(16, 3, 3, 3, 224, 224),
        strides=(s[0], s[1], s[2], s[3], s[2], s[3]))
    # rows (ky, kx, c) to match w1 layout -> [16, 27, 224, 224]
    imcol = win.transpose(0, 2, 3, 1, 4, 5).reshape(16, 27, 224, 224)
    # split rows into 4 groups of 56, append the ones row (bias via K)
    imgq = np.ones((16, 4, 28, 56 * 224), np.float32)
    imgq[:, :, 0:27, :] = imcol.reshape(16, 27, 4, 56 * 224).transpose(0, 2, 1, 3)
    imgq = imgq.astype(ml_dtypes.bfloat16)
    in_maps = []
    for c in range(NCORES):
        caps = captions[BL * c:BL * (c + 1)].astype(np.int64).T.reshape(NTOK, 1)
        m = dict(shared)
        m["img"] = imgq[BL * c:BL * (c + 1)].copy()
        m["caps"] = caps.astype(np.int32)
        in_maps.append(m)
    return in_maps


def kernel(**inputs):
    from concourse.bass_utils import run_bass_kernel_spmd

    if "nc" not in _NC_CACHE:
        _NC_CACHE["nc"] = build_bass()
    nc = _NC_CACHE["nc"]

    in_maps = _make_in_maps(inputs)
    res = run_bass_kernel_spmd(nc, in_maps, list(range(NCORES)))
    out = np.concatenate([res.results[c]["logits"] for c in range(NCORES)], axis=0)
    return out
